# revision 18
# baseline (speedup 1.0000x reference)
"""Group MoE layer (2 groups x 4 experts, top-1 group / top-2 expert routing)
on 8 Trainium2 NeuronCores via expert parallelism.

Strategy:
  - Host computes the (tiny) routing: language-gate argmax over groups,
    per-group expert top-k + softmax weights.
  - Tokens are dispatched by (group, expert) assignment: core c = g*4+e
    receives exactly the tokens routed to expert (g, e), padded to a common
    capacity C (SPMD: all cores run the same program).
  - Each core runs the dense FFN for its expert:
        Y^T = W2 @ relu(W1 @ X^T + b1) + b2      (tokens in the moving dim)
    with bf16 weights/activations and fp32 PSUM accumulation.
  - The capacity remainder above a multiple of 512 (e.g. C=1051 -> 2x512+27)
    does NOT get its own pass over the weights: a separate pass costs a full
    LDWEIGHTS sweep (512 weight tiles x ~56ns ~= 29us) regardless of token
    count. Instead the remainder rides the first full block's weight stream:
    each weight tile issues a second matmul (N=rem) reusing the stationary
    operand, costing only ~14ns extra per tile (~7us total).
  - DMA: merged tiles (one dma_start each, 128 large descriptors); the first
    W1 chunks are small so the PE can start as soon as x block 0 plus one
    h-tile of W1 land; critical-path loads are split across the two HWDGE
    queues (sync + scalar).
  - Host scatter-adds the weighted expert outputs back into the full output.
"""

import numpy as np
import ml_dtypes

import concourse.bacc as bacc
import concourse.mybir as mybir
from concourse import tile
from concourse import bass_utils

B, L, D, H = 2, 2048, 1024, 4096
G, E = 2, 4
NCORES = G * E
PART = 128
TOK_BLK = 512
W2GRP = 4                       # h-tiles per merged W2 tile (8KB rows, 1MB)
# Variable W1 chunk widths (columns of H per DMA). Chunks below 512 columns
# (8KB per partition row) transfer at a fraction of line rate (measured:
# 2KB rows ~34GB/s vs 8KB rows ~400GB/s), so 512 is the floor; the first
# chunk is the smallest so the PE start only gates on 1MB of W1.
W1CHUNKS = (512, 512, 1024, 1024, 1024)
assert sum(W1CHUNKS) == H

_BF16 = ml_dtypes.bfloat16

_program_cache: dict[tuple, object] = {}


def _build(nfull: int, rem: int, rem_pad: int, d: int = D, h: int = H):
    """Per-core expert FFN program: nfull token blocks of 512 plus an
    optional remainder of `rem` tokens merged into pass 0's weight stream."""
    key = (nfull, rem, rem_pad, d, h)
    if key in _program_cache:
        return _program_cache[key]

    nd = d // PART
    nh = h // PART
    ng2 = nh // W2GRP
    # chunk index + h-tile offset within chunk, per h-tile
    chunk_of = []
    h0 = 0
    for c, w in enumerate(W1CHUNKS):
        for off in range(w // PART):
            chunk_of.append((c, off))
        h0 += w
    assert len(chunk_of) == nh

    bf16 = mybir.dt.bfloat16
    f32 = mybir.dt.float32

    nc = bacc.Bacc("TRN2", target_bir_lowering=False, debug=False,
                   num_devices=NCORES)

    xt = nc.dram_tensor("xt", [nfull, PART, nd * TOK_BLK], bf16,
                        kind="ExternalInput")
    w1c = [nc.dram_tensor(f"w1c{c}", [PART, nd * w], bf16,
                          kind="ExternalInput")
           for c, w in enumerate(W1CHUNKS)]
    w2t = nc.dram_tensor("w2t", [ng2, PART, W2GRP * d], bf16,
                         kind="ExternalInput")
    b1t = nc.dram_tensor("b1t", [PART, nh], f32, kind="ExternalInput")
    b2t = nc.dram_tensor("b2t", [PART, nd], f32, kind="ExternalInput")
    yt = nc.dram_tensor("yt", [nfull, PART, nd * TOK_BLK], f32,
                        kind="ExternalOutput")
    if rem:
        xr = nc.dram_tensor("xr", [PART, nd * rem_pad], bf16,
                            kind="ExternalInput")
        yr = nc.dram_tensor("yr", [PART, nd * rem_pad], f32,
                            kind="ExternalOutput")

    with tile.TileContext(nc) as tc:
        with (
            tc.tile_pool(name="wpool", bufs=1) as wpool,
            tc.tile_pool(name="h1pool", bufs=nh) as h1pool,
            tc.tile_pool(name="ypool", bufs=1) as ypool,
            tc.tile_pool(name="ps1", bufs=2, space="PSUM") as ps1,
            tc.tile_pool(name="ps2", bufs=2, space="PSUM") as ps2,
        ):
            # --- loads, in consumption order ------------------------------
            # scalar HWDGE queue: first W1 h-tile (gates the first matmul),
            # biases (needed by the first relu / bias add), second W1 h-tile,
            # then the bulk W2 (needed only when mm2 of pass 0 starts).
            w1_sb = [None] * len(W1CHUNKS)

            def load_w1(c, eng):
                t = wpool.tile([PART, nd * W1CHUNKS[c]], bf16, tag=f"w1_{c}")
                eng.dma_start(out=t[:, :], in_=w1c[c].ap()[:, :])
                w1_sb[c] = t

            # Load scheduling constraints (all HW-measured on this part):
            #  - transfers whose per-partition rows are < 8KB crawl once the
            #    PE is streaming (2KB rows: ~34GB/s), so every bulk tensor is
            #    packed with >= 8KB rows and the tiny ones (b1/b2/xr) sit
            #    where their crawl cannot delay a bulk load;
            #  - the two HWDGE rings (sync/scalar) share HBM bandwidth, so
            #    W1 (needed progressively from t~15us) gets a dedicated ring
            #    while x0/biases/W2 share the other;
            #  - mm2's reversed accumulation order (below) keeps W2 off the
            #    critical path entirely (deadline ~85us, lands ~55us).
            # CRITICAL: the scalar engine also executes the activations, and
            # a HWDGE trigger blocks its FIFO once the ring saturates (~4+
            # queued transfers) — queuing the bulk stream there deadlocks
            # the ACT->PSUM-free->matmul chain for tens of us (measured).
            # So the scalar ring gets exactly 4 small triggers (x block 0,
            # rem x, biases) and EVERYTHING else — W1 in consumption order,
            # then W2, then the later x blocks — serializes on the sync
            # ring, whose engine has nothing else to do.
            x_sb = [None] * nfull

            def load_x(blk, eng):
                t = wpool.tile([PART, nd * TOK_BLK], bf16, tag=f"x_{blk}")
                eng.dma_start(out=t[:, :], in_=xt.ap()[blk])
                x_sb[blk] = t

            # PE warm-up: the HAM clock gate holds the PE at 1.2GHz until it
            # has been busy ~3.4us; ~60 throwaway matmuls on a zeroed scratch
            # tile (no DMA deps) keep the PE busy during the initial load so
            # the real stream starts at 2.4GHz.
            warm_x = wpool.tile([PART, TOK_BLK], bf16, tag="warm")
            nc.gpsimd.memset(warm_x[:, :], 0.0)
            warm_ps = ps1.tile([PART, TOK_BLK], f32, tag="m")
            for _ in range(60):
                nc.tensor.matmul(warm_ps[:, :], warm_x[:, :PART],
                                 warm_x[:, :], start=True, stop=True)

            load_w1(0, nc.sync)
            load_x(0, nc.scalar)
            b1_sb = wpool.tile([PART, nh], f32, tag="b1")
            nc.scalar.dma_start(out=b1_sb[:, :], in_=b1t.ap()[:, :])
            if rem:
                xr_sb = wpool.tile([PART, nd * rem_pad], bf16, tag="xr")
                nc.scalar.dma_start(out=xr_sb[:, :], in_=xr.ap()[:, :])
            for c in range(1, len(W1CHUNKS)):
                load_w1(c, nc.sync)

            w2_sb = [None] * ng2
            for gi in range(ng2):
                t = wpool.tile([PART, W2GRP * d], bf16, tag=f"w2_{gi}")
                nc.sync.dma_start(out=t[:, :], in_=w2t.ap()[gi])
                w2_sb[gi] = t
            for blk in range(1, nfull):
                load_x(blk, nc.sync)
            # b2 is only read by the bias adds in mm2 (~80us in): last on sync
            b2_sb = wpool.tile([PART, nd], f32, tag="b2")
            nc.sync.dma_start(out=b2_sb[:, :], in_=b2t.ap()[:, :])

            # --- compute passes ------------------------------------------
            for p in range(nfull):
                merged = (p == 0 and rem > 0)
                h1m_tiles = []
                h1r_tiles = []
                for hi in range(nh):
                    c, off = chunk_of[hi]
                    wch = W1CHUNKS[c]
                    psm = ps1.tile([PART, TOK_BLK], f32, tag="m")
                    if merged:
                        psr = ps1.tile([PART, TOK_BLK], f32, tag="r")
                    for di in range(nd):
                        w_ap = w1_sb[c][:, di * wch + off * PART:
                                        di * wch + (off + 1) * PART]
                        nc.tensor.matmul(
                            psm[:, :], w_ap,
                            x_sb[p][:, di * TOK_BLK:(di + 1) * TOK_BLK],
                            start=(di == 0), stop=(di == nd - 1),
                        )
                        if merged:
                            # second matmul on the same stationary weights:
                            # the LDWEIGHTS is shared, ~14ns marginal cost
                            nc.tensor.matmul(
                                psr[:, :rem], w_ap,
                                xr_sb[:, di * rem_pad:di * rem_pad + rem],
                                start=(di == 0), stop=(di == nd - 1),
                            )
                    h1m = h1pool.tile([PART, TOK_BLK], bf16, tag="h1m")
                    nc.scalar.activation(h1m[:, :], psm[:, :],
                                         mybir.ActivationFunctionType.Relu,
                                         bias=b1_sb[:, hi:hi + 1], scale=1.0)
                    h1m_tiles.append(h1m)
                    if merged:
                        h1r = h1pool.tile([PART, rem_pad], bf16, tag="h1r")
                        nc.scalar.activation(
                            h1r[:, :rem], psr[:, :rem],
                            mybir.ActivationFunctionType.Relu,
                            bias=b1_sb[:, hi:hi + 1], scale=1.0)
                        h1r_tiles.append(h1r)

                y = ypool.tile([PART, nd * TOK_BLK], f32, tag="y")
                if merged:
                    y_r = ypool.tile([PART, nd * rem_pad], f32, tag="yr")
                for di in range(nd):
                    ps2m = ps2.tile([PART, TOK_BLK], f32, tag="m")
                    if merged:
                        ps2r = ps2.tile([PART, TOK_BLK], f32, tag="r")
                    # hi runs high->low: the chain's first matmul then needs
                    # the LAST h1 tile, so the compile-time scheduler cannot
                    # hoist mm2 matmuls (whose W2 may still be in flight)
                    # ahead of ready mm1 work in the in-order PE queue.
                    for hi in range(nh - 1, -1, -1):
                        gi, hj = divmod(hi, W2GRP)
                        w_ap = w2_sb[gi][:, hj * d + di * PART:
                                         hj * d + (di + 1) * PART]
                        nc.tensor.matmul(
                            ps2m[:, :], w_ap, h1m_tiles[hi][:, :],
                            start=(hi == nh - 1), stop=(hi == 0),
                        )
                        if merged:
                            nc.tensor.matmul(
                                ps2r[:, :rem], w_ap, h1r_tiles[hi][:, :rem],
                                start=(hi == nh - 1), stop=(hi == 0),
                            )
                    nc.vector.tensor_scalar_add(
                        y[:, di * TOK_BLK:(di + 1) * TOK_BLK], ps2m[:, :],
                        b2_sb[:, di:di + 1])
                    # drain several d-tiles per DMA (8KB rows go at line
                    # rate, per-tile 2KB rows crawl under compute), but keep
                    # the final drain a single d-tile so the post-last-matmul
                    # tail transfer is small
                    if di in (3, nd - 2, nd - 1):
                        lo = 0 if di == 3 else (4 if di == nd - 2 else nd - 1)
                        nc.sync.dma_start(
                            out=yt.ap()[p][:, lo * TOK_BLK:(di + 1) * TOK_BLK],
                            in_=y[:, lo * TOK_BLK:(di + 1) * TOK_BLK])
                    if merged:
                        nc.vector.tensor_scalar_add(
                            y_r[:, di * rem_pad:di * rem_pad + rem],
                            ps2r[:, :rem], b2_sb[:, di:di + 1])
                if merged:
                    # one drain for the whole remainder block (mid-kernel,
                    # fully overlapped with pass 1's compute)
                    nc.sync.dma_start(out=yr.ap()[:, :], in_=y_r[:, :])

    nc.compile()
    _program_cache[key] = nc
    return nc


def _route(x, bn, Wlg, blg, Wg, k):
    """Numpy replica of the reference routing. Returns per-(g,e) assignment."""
    glog = bn @ Wlg.T + blg                       # (N, G)
    sel_group = np.argmax(glog, axis=1)           # (N,)
    assign = []
    for g in range(Wg.shape[0]):
        logits = x @ Wg[g].T                      # (N, E)
        order = np.argsort(-logits, axis=1, kind="stable")
        sel = order[:, :k]                        # (N, k)
        top = np.take_along_axis(logits, sel, axis=1).astype(np.float32)
        m = top.max(axis=1, keepdims=True)
        ex = np.exp(top - m)
        w = ex / ex.sum(axis=1, keepdims=True)    # (N, k)
        assign.append((sel, w))
    return sel_group, assign


def _pack_x(X, d, nblk, tok_blk):
    """(nblk*tok_blk, d) fp32 -> [nblk, 128, nd*tok_blk] bf16 merged tiles."""
    nd = d // PART
    xt = X.T.astype(_BF16)                        # (d, nblk*tok_blk)
    return np.ascontiguousarray(
        xt.reshape(nd, PART, nblk, tok_blk).transpose(2, 1, 0, 3)
          .reshape(nblk, PART, nd * tok_blk))


def _pack_w1_chunk(W1e_T, d, h0, w):
    """W1e.T slice (d, h0:h0+w) fp32 -> [128, nd*w] bf16."""
    nd = d // PART
    wsl = W1e_T[:, h0:h0 + w].astype(_BF16)       # (d, w)
    return np.ascontiguousarray(
        wsl.reshape(nd, PART, w).transpose(1, 0, 2).reshape(PART, nd * w))


def _pack_w2(W2e, d, h):
    ng2 = h // PART // W2GRP
    w = W2e.T.astype(_BF16)                       # (h, d)
    return np.ascontiguousarray(
        w.reshape(ng2, W2GRP, PART, d).transpose(0, 2, 1, 3)
         .reshape(ng2, PART, W2GRP * d))


def _unpack_y(yt, d, nblk, tok_blk):
    """[nblk, 128, nd*tok_blk] f32 -> (d, nblk*tok_blk)."""
    nd = d // PART
    return (yt.reshape(nblk, PART, nd, tok_blk).transpose(2, 1, 0, 3)
              .reshape(d, nblk * tok_blk))


def kernel(**inputs) -> np.ndarray:
    xs = np.asarray(inputs["xs"], np.float32)
    bn = np.asarray(inputs["bottle_neck"], np.float32)
    Wlg = np.asarray(inputs["Wlg"], np.float32)
    blg = np.asarray(inputs["blg"], np.float32)
    Wg = np.asarray(inputs["Wg"], np.float32)
    W1 = np.asarray(inputs["W1"], np.float32)
    b1 = np.asarray(inputs["b1"], np.float32)
    W2 = np.asarray(inputs["W2"], np.float32)
    b2 = np.asarray(inputs["b2"], np.float32)
    k = int(np.asarray(inputs["top_k"]))

    Bx, Lx, d = xs.shape
    hdim = W1.shape[2]
    N = Bx * Lx
    nh = hdim // PART
    nd = d // PART
    x = xs.reshape(N, d)
    bnf = bn.reshape(N, d)

    sel_group, assign = _route(x, bnf, Wlg, blg, Wg, k)

    # Token sets per (group, expert) core.
    idxs, wgts = [], []
    for c in range(NCORES):
        g, e = divmod(c, E)
        sel, w = assign[g]
        mask = (sel_group == g)[:, None] & (sel == e)
        rows, cols = np.nonzero(mask)
        idxs.append(rows)
        wgts.append(w[rows, cols])

    cnt_max = max(len(i) for i in idxs)
    nfull = max(1, cnt_max // TOK_BLK)
    rem = cnt_max - nfull * TOK_BLK
    if rem < 0:                                   # cnt_max < 512
        rem = 0
    rem_pad = -(-rem // 32) * 32 if rem else 0
    C_pad = nfull * TOK_BLK + rem_pad

    nc = _build(nfull, rem, rem_pad, d, hdim)

    h_offsets = np.concatenate(([0], np.cumsum(W1CHUNKS)))[:-1]
    in_maps = []
    for c in range(NCORES):
        g, e = divmod(c, E)
        cnt = len(idxs[c])
        X = np.zeros((C_pad, d), np.float32)
        X[:cnt] = x[idxs[c]]
        w1T = W1[g, e].T                          # (d, h)
        m = {
            "xt": _pack_x(X[:nfull * TOK_BLK], d, nfull, TOK_BLK),
            "w2t": _pack_w2(W2[g, e], d, hdim),
            "b1t": np.ascontiguousarray(b1[g, e].reshape(nh, PART).T),
            "b2t": np.ascontiguousarray(b2[g, e].reshape(nd, PART).T),
        }
        for ci, w in enumerate(W1CHUNKS):
            m[f"w1c{ci}"] = _pack_w1_chunk(w1T, d, int(h_offsets[ci]), w)
        if rem:
            m["xr"] = _pack_x(X[nfull * TOK_BLK:], d, 1, rem_pad)[0]
        in_maps.append(m)

    res = bass_utils.run_bass_kernel_spmd(nc, in_maps, core_ids=list(range(NCORES)))

    out = np.zeros((N, d), np.float32)
    for c in range(NCORES):
        cnt = len(idxs[c])
        if cnt == 0:
            continue
        y_full = _unpack_y(res.results[c]["yt"], d, nfull, TOK_BLK)
        if rem:
            y_rem = _unpack_y(res.results[c]["yr"], d, 1, rem_pad)
            y_full = np.concatenate([y_full, y_rem], axis=1)
        yc = y_full[:, :cnt].T
        out[idxs[c]] += wgts[c][:, None] * yc
    return out.reshape(Bx, Lx, d).astype(np.float32)


# revision 19
# speedup vs baseline: 1.0036x; 1.0036x over previous
"""Group MoE layer (2 groups x 4 experts, top-1 group / top-2 expert routing)
on 8 Trainium2 NeuronCores via expert parallelism.

Strategy:
  - Host computes the (tiny) routing: language-gate argmax over groups,
    per-group expert top-k + softmax weights.
  - Tokens are dispatched by (group, expert) assignment: core c = g*4+e
    receives exactly the tokens routed to expert (g, e), padded to a common
    capacity C (SPMD: all cores run the same program).
  - Each core runs the dense FFN for its expert:
        Y^T = W2 @ relu(W1 @ X^T + b1) + b2      (tokens in the moving dim)
    with bf16 weights/activations and fp32 PSUM accumulation.
  - The capacity remainder above a multiple of 512 (e.g. C=1051 -> 2x512+27)
    does NOT get its own pass over the weights: a separate pass costs a full
    LDWEIGHTS sweep (512 weight tiles x ~56ns ~= 29us) regardless of token
    count. Instead the remainder rides the first full block's weight stream:
    each weight tile issues a second matmul (N=rem) reusing the stationary
    operand, costing only ~14ns extra per tile (~7us total).
  - DMA: merged tiles (one dma_start each, 128 large descriptors); the first
    W1 chunks are small so the PE can start as soon as x block 0 plus one
    h-tile of W1 land; critical-path loads are split across the two HWDGE
    queues (sync + scalar).
  - Host scatter-adds the weighted expert outputs back into the full output.
"""

import numpy as np
import ml_dtypes

import concourse.bacc as bacc
import concourse.mybir as mybir
from concourse import tile
from concourse import bass_utils

B, L, D, H = 2, 2048, 1024, 4096
G, E = 2, 4
NCORES = G * E
PART = 128
TOK_BLK = 512
W2GRP = 4                       # h-tiles per merged W2 tile (8KB rows, 1MB)
# Variable W1 chunk widths (columns of H per DMA). Chunks below 512 columns
# (8KB per partition row) transfer at a fraction of line rate (measured:
# 2KB rows ~34GB/s vs 8KB rows ~400GB/s), so 512 is the floor; the first
# chunk is the smallest so the PE start only gates on 1MB of W1.
W1CHUNKS = (512, 512, 1024, 1024, 1024)
assert sum(W1CHUNKS) == H

_BF16 = ml_dtypes.bfloat16

_program_cache: dict[tuple, object] = {}


def _build(nfull: int, rem: int, rem_pad: int, d: int = D, h: int = H):
    """Per-core expert FFN program: nfull token blocks of 512 plus an
    optional remainder of `rem` tokens merged into pass 0's weight stream."""
    key = (nfull, rem, rem_pad, d, h)
    if key in _program_cache:
        return _program_cache[key]

    nd = d // PART
    nh = h // PART
    ng2 = nh // W2GRP
    # chunk index + h-tile offset within chunk, per h-tile
    chunk_of = []
    h0 = 0
    for c, w in enumerate(W1CHUNKS):
        for off in range(w // PART):
            chunk_of.append((c, off))
        h0 += w
    assert len(chunk_of) == nh

    bf16 = mybir.dt.bfloat16
    f32 = mybir.dt.float32

    nc = bacc.Bacc("TRN2", target_bir_lowering=False, debug=False,
                   num_devices=NCORES)

    xt = nc.dram_tensor("xt", [nfull, PART, nd * TOK_BLK], bf16,
                        kind="ExternalInput")
    w1c = [nc.dram_tensor(f"w1c{c}", [PART, nd * w], bf16,
                          kind="ExternalInput")
           for c, w in enumerate(W1CHUNKS)]
    w2t = nc.dram_tensor("w2t", [ng2, PART, W2GRP * d], bf16,
                         kind="ExternalInput")
    b1t = nc.dram_tensor("b1t", [PART, nh], f32, kind="ExternalInput")
    b2t = nc.dram_tensor("b2t", [PART, nd], f32, kind="ExternalInput")
    yt = nc.dram_tensor("yt", [nfull, PART, nd * TOK_BLK], f32,
                        kind="ExternalOutput")
    if rem:
        xr = nc.dram_tensor("xr", [PART, nd * rem_pad], bf16,
                            kind="ExternalInput")
        yr = nc.dram_tensor("yr", [PART, nd * rem_pad], f32,
                            kind="ExternalOutput")

    with tile.TileContext(nc) as tc:
        with (
            tc.tile_pool(name="wpool", bufs=1) as wpool,
            tc.tile_pool(name="h1pool", bufs=nh) as h1pool,
            tc.tile_pool(name="ypool", bufs=1) as ypool,
            tc.tile_pool(name="ps1", bufs=2, space="PSUM") as ps1,
            tc.tile_pool(name="ps2", bufs=2, space="PSUM") as ps2,
        ):
            # --- loads, in consumption order ------------------------------
            # scalar HWDGE queue: first W1 h-tile (gates the first matmul),
            # biases (needed by the first relu / bias add), second W1 h-tile,
            # then the bulk W2 (needed only when mm2 of pass 0 starts).
            w1_sb = [None] * len(W1CHUNKS)

            def load_w1(c, eng):
                t = wpool.tile([PART, nd * W1CHUNKS[c]], bf16, tag=f"w1_{c}")
                eng.dma_start(out=t[:, :], in_=w1c[c].ap()[:, :])
                w1_sb[c] = t

            # Load scheduling constraints (all HW-measured on this part):
            #  - transfers whose per-partition rows are < 8KB crawl once the
            #    PE is streaming (2KB rows: ~34GB/s), so every bulk tensor is
            #    packed with >= 8KB rows and the tiny ones (b1/b2/xr) sit
            #    where their crawl cannot delay a bulk load;
            #  - the two HWDGE rings (sync/scalar) share HBM bandwidth, so
            #    W1 (needed progressively from t~15us) gets a dedicated ring
            #    while x0/biases/W2 share the other;
            #  - mm2's reversed accumulation order (below) keeps W2 off the
            #    critical path entirely (deadline ~85us, lands ~55us).
            # CRITICAL: the scalar engine also executes the activations, and
            # a HWDGE trigger blocks its FIFO once the ring saturates (~4+
            # queued transfers) — queuing the bulk stream there deadlocks
            # the ACT->PSUM-free->matmul chain for tens of us (measured).
            # So the scalar ring gets exactly 4 small triggers (x block 0,
            # rem x, biases) and EVERYTHING else — W1 in consumption order,
            # then W2, then the later x blocks — serializes on the sync
            # ring, whose engine has nothing else to do.
            x_sb = [None] * nfull

            def load_x(blk, eng):
                t = wpool.tile([PART, nd * TOK_BLK], bf16, tag=f"x_{blk}")
                eng.dma_start(out=t[:, :], in_=xt.ap()[blk])
                x_sb[blk] = t

            # PE warm-up: the HAM clock gate holds the PE at 1.2GHz until it
            # has been busy ~3.4us; ~60 throwaway matmuls on a zeroed scratch
            # tile (no DMA deps) keep the PE busy during the initial load so
            # the real stream starts at 2.4GHz.
            warm_x = wpool.tile([PART, TOK_BLK], bf16, tag="warm")
            nc.gpsimd.memset(warm_x[:, :], 0.0)
            warm_ps = ps1.tile([PART, TOK_BLK], f32, tag="m")
            for _ in range(40):
                nc.tensor.matmul(warm_ps[:, :], warm_x[:, :PART],
                                 warm_x[:, :], start=True, stop=True)

            load_w1(0, nc.sync)
            load_x(0, nc.scalar)
            b1_sb = wpool.tile([PART, nh], f32, tag="b1")
            nc.scalar.dma_start(out=b1_sb[:, :], in_=b1t.ap()[:, :])
            if rem:
                xr_sb = wpool.tile([PART, nd * rem_pad], bf16, tag="xr")
                nc.scalar.dma_start(out=xr_sb[:, :], in_=xr.ap()[:, :])
            for c in range(1, len(W1CHUNKS)):
                load_w1(c, nc.sync)

            w2_sb = [None] * ng2
            for gi in range(ng2):
                t = wpool.tile([PART, W2GRP * d], bf16, tag=f"w2_{gi}")
                nc.sync.dma_start(out=t[:, :], in_=w2t.ap()[gi])
                w2_sb[gi] = t
            for blk in range(1, nfull):
                load_x(blk, nc.sync)
            # b2 is only read by the bias adds in mm2 (~80us in): last on sync
            b2_sb = wpool.tile([PART, nd], f32, tag="b2")
            nc.sync.dma_start(out=b2_sb[:, :], in_=b2t.ap()[:, :])

            # --- compute passes ------------------------------------------
            for p in range(nfull):
                merged = (p == 0 and rem > 0)
                h1m_tiles = []
                h1r_tiles = []
                for hi in range(nh):
                    c, off = chunk_of[hi]
                    wch = W1CHUNKS[c]
                    psm = ps1.tile([PART, TOK_BLK], f32, tag="m")
                    if merged:
                        psr = ps1.tile([PART, TOK_BLK], f32, tag="r")
                    for di in range(nd):
                        w_ap = w1_sb[c][:, di * wch + off * PART:
                                        di * wch + (off + 1) * PART]
                        nc.tensor.matmul(
                            psm[:, :], w_ap,
                            x_sb[p][:, di * TOK_BLK:(di + 1) * TOK_BLK],
                            start=(di == 0), stop=(di == nd - 1),
                        )
                        if merged:
                            # second matmul on the same stationary weights:
                            # the LDWEIGHTS is shared, ~14ns marginal cost
                            nc.tensor.matmul(
                                psr[:, :rem], w_ap,
                                xr_sb[:, di * rem_pad:di * rem_pad + rem],
                                start=(di == 0), stop=(di == nd - 1),
                            )
                    h1m = h1pool.tile([PART, TOK_BLK], bf16, tag="h1m")
                    nc.scalar.activation(h1m[:, :], psm[:, :],
                                         mybir.ActivationFunctionType.Relu,
                                         bias=b1_sb[:, hi:hi + 1], scale=1.0)
                    h1m_tiles.append(h1m)
                    if merged:
                        h1r = h1pool.tile([PART, rem_pad], bf16, tag="h1r")
                        nc.scalar.activation(
                            h1r[:, :rem], psr[:, :rem],
                            mybir.ActivationFunctionType.Relu,
                            bias=b1_sb[:, hi:hi + 1], scale=1.0)
                        h1r_tiles.append(h1r)

                y = ypool.tile([PART, nd * TOK_BLK], f32, tag="y")
                if merged:
                    y_r = ypool.tile([PART, nd * rem_pad], f32, tag="yr")
                for di in range(nd):
                    ps2m = ps2.tile([PART, TOK_BLK], f32, tag="m")
                    if merged:
                        ps2r = ps2.tile([PART, TOK_BLK], f32, tag="r")
                    # hi runs high->low: the chain's first matmul then needs
                    # the LAST h1 tile, so the compile-time scheduler cannot
                    # hoist mm2 matmuls (whose W2 may still be in flight)
                    # ahead of ready mm1 work in the in-order PE queue.
                    for hi in range(nh - 1, -1, -1):
                        gi, hj = divmod(hi, W2GRP)
                        w_ap = w2_sb[gi][:, hj * d + di * PART:
                                         hj * d + (di + 1) * PART]
                        nc.tensor.matmul(
                            ps2m[:, :], w_ap, h1m_tiles[hi][:, :],
                            start=(hi == nh - 1), stop=(hi == 0),
                        )
                        if merged:
                            nc.tensor.matmul(
                                ps2r[:, :rem], w_ap, h1r_tiles[hi][:, :rem],
                                start=(hi == nh - 1), stop=(hi == 0),
                            )
                    nc.vector.tensor_scalar_add(
                        y[:, di * TOK_BLK:(di + 1) * TOK_BLK], ps2m[:, :],
                        b2_sb[:, di:di + 1])
                    # drain several d-tiles per DMA (8KB rows go at line
                    # rate, per-tile 2KB rows crawl under compute), but keep
                    # the final drain a single d-tile so the post-last-matmul
                    # tail transfer is small
                    if di in (3, nd - 2, nd - 1):
                        lo = 0 if di == 3 else (4 if di == nd - 2 else nd - 1)
                        nc.sync.dma_start(
                            out=yt.ap()[p][:, lo * TOK_BLK:(di + 1) * TOK_BLK],
                            in_=y[:, lo * TOK_BLK:(di + 1) * TOK_BLK])
                    if merged:
                        nc.vector.tensor_scalar_add(
                            y_r[:, di * rem_pad:di * rem_pad + rem],
                            ps2r[:, :rem], b2_sb[:, di:di + 1])
                if merged:
                    # one drain for the whole remainder block (mid-kernel,
                    # fully overlapped with pass 1's compute)
                    nc.sync.dma_start(out=yr.ap()[:, :], in_=y_r[:, :])

    nc.compile()
    _program_cache[key] = nc
    return nc


def _route(x, bn, Wlg, blg, Wg, k):
    """Numpy replica of the reference routing. Returns per-(g,e) assignment."""
    glog = bn @ Wlg.T + blg                       # (N, G)
    sel_group = np.argmax(glog, axis=1)           # (N,)
    assign = []
    for g in range(Wg.shape[0]):
        logits = x @ Wg[g].T                      # (N, E)
        order = np.argsort(-logits, axis=1, kind="stable")
        sel = order[:, :k]                        # (N, k)
        top = np.take_along_axis(logits, sel, axis=1).astype(np.float32)
        m = top.max(axis=1, keepdims=True)
        ex = np.exp(top - m)
        w = ex / ex.sum(axis=1, keepdims=True)    # (N, k)
        assign.append((sel, w))
    return sel_group, assign


def _pack_x(X, d, nblk, tok_blk):
    """(nblk*tok_blk, d) fp32 -> [nblk, 128, nd*tok_blk] bf16 merged tiles."""
    nd = d // PART
    xt = X.T.astype(_BF16)                        # (d, nblk*tok_blk)
    return np.ascontiguousarray(
        xt.reshape(nd, PART, nblk, tok_blk).transpose(2, 1, 0, 3)
          .reshape(nblk, PART, nd * tok_blk))


def _pack_w1_chunk(W1e_T, d, h0, w):
    """W1e.T slice (d, h0:h0+w) fp32 -> [128, nd*w] bf16."""
    nd = d // PART
    wsl = W1e_T[:, h0:h0 + w].astype(_BF16)       # (d, w)
    return np.ascontiguousarray(
        wsl.reshape(nd, PART, w).transpose(1, 0, 2).reshape(PART, nd * w))


def _pack_w2(W2e, d, h):
    ng2 = h // PART // W2GRP
    w = W2e.T.astype(_BF16)                       # (h, d)
    return np.ascontiguousarray(
        w.reshape(ng2, W2GRP, PART, d).transpose(0, 2, 1, 3)
         .reshape(ng2, PART, W2GRP * d))


def _unpack_y(yt, d, nblk, tok_blk):
    """[nblk, 128, nd*tok_blk] f32 -> (d, nblk*tok_blk)."""
    nd = d // PART
    return (yt.reshape(nblk, PART, nd, tok_blk).transpose(2, 1, 0, 3)
              .reshape(d, nblk * tok_blk))


def kernel(**inputs) -> np.ndarray:
    xs = np.asarray(inputs["xs"], np.float32)
    bn = np.asarray(inputs["bottle_neck"], np.float32)
    Wlg = np.asarray(inputs["Wlg"], np.float32)
    blg = np.asarray(inputs["blg"], np.float32)
    Wg = np.asarray(inputs["Wg"], np.float32)
    W1 = np.asarray(inputs["W1"], np.float32)
    b1 = np.asarray(inputs["b1"], np.float32)
    W2 = np.asarray(inputs["W2"], np.float32)
    b2 = np.asarray(inputs["b2"], np.float32)
    k = int(np.asarray(inputs["top_k"]))

    Bx, Lx, d = xs.shape
    hdim = W1.shape[2]
    N = Bx * Lx
    nh = hdim // PART
    nd = d // PART
    x = xs.reshape(N, d)
    bnf = bn.reshape(N, d)

    sel_group, assign = _route(x, bnf, Wlg, blg, Wg, k)

    # Token sets per (group, expert) core.
    idxs, wgts = [], []
    for c in range(NCORES):
        g, e = divmod(c, E)
        sel, w = assign[g]
        mask = (sel_group == g)[:, None] & (sel == e)
        rows, cols = np.nonzero(mask)
        idxs.append(rows)
        wgts.append(w[rows, cols])

    cnt_max = max(len(i) for i in idxs)
    nfull = max(1, cnt_max // TOK_BLK)
    rem = cnt_max - nfull * TOK_BLK
    if rem < 0:                                   # cnt_max < 512
        rem = 0
    rem_pad = -(-rem // 32) * 32 if rem else 0
    C_pad = nfull * TOK_BLK + rem_pad

    nc = _build(nfull, rem, rem_pad, d, hdim)

    h_offsets = np.concatenate(([0], np.cumsum(W1CHUNKS)))[:-1]
    in_maps = []
    for c in range(NCORES):
        g, e = divmod(c, E)
        cnt = len(idxs[c])
        X = np.zeros((C_pad, d), np.float32)
        X[:cnt] = x[idxs[c]]
        w1T = W1[g, e].T                          # (d, h)
        m = {
            "xt": _pack_x(X[:nfull * TOK_BLK], d, nfull, TOK_BLK),
            "w2t": _pack_w2(W2[g, e], d, hdim),
            "b1t": np.ascontiguousarray(b1[g, e].reshape(nh, PART).T),
            "b2t": np.ascontiguousarray(b2[g, e].reshape(nd, PART).T),
        }
        for ci, w in enumerate(W1CHUNKS):
            m[f"w1c{ci}"] = _pack_w1_chunk(w1T, d, int(h_offsets[ci]), w)
        if rem:
            m["xr"] = _pack_x(X[nfull * TOK_BLK:], d, 1, rem_pad)[0]
        in_maps.append(m)

    res = bass_utils.run_bass_kernel_spmd(nc, in_maps, core_ids=list(range(NCORES)))

    out = np.zeros((N, d), np.float32)
    for c in range(NCORES):
        cnt = len(idxs[c])
        if cnt == 0:
            continue
        y_full = _unpack_y(res.results[c]["yt"], d, nfull, TOK_BLK)
        if rem:
            y_rem = _unpack_y(res.results[c]["yr"], d, 1, rem_pad)
            y_full = np.concatenate([y_full, y_rem], axis=1)
        yc = y_full[:, :cnt].T
        out[idxs[c]] += wgts[c][:, None] * yc
    return out.reshape(Bx, Lx, d).astype(np.float32)
